# revision 31
# baseline (speedup 1.0000x reference)
"""Distributed Trainium2 kernel for nn_Attention_31370441130243.

Full-input / full-output attention layer, sharded internally over the
8 NeuronCores as (batch=2) x (head-group=4): core c handles batch c//4
and heads [4*(c%4), 4*(c%4)+4).  Each core computes its QKV projections,
per-head RMSNorm + RoPE, non-causal SDPA and a partial output projection
(its Wout column block); the host sums the 4 partials per batch.

v3 design (all 16-bit tensors are fp16; PSUM accumulation is fp32):
  - One pass over x computes q, k, v per 128-row s-block (j): the three
    matmuls share each stationary x tile, so x streams from HBM once.
  - RMSNorm: 1/(rms*C^0.25) is applied to BOTH q and k (scalar-engine
    copy-with-per-partition-scale), so the score exp needs no scale.
  - RoPE tables are pre-broadened to [128, 512] on GpSimd so the DVE
    combine is three flat 1-D ops; PE transposes in fp16 are deferred
    into the next block's matmul stream to hide the elementwise chain.
  - Scores are computed transposed (scT = kT_blk.T @ qT = [t, s]) so the
    PV matmul needs no P transpose; exp -> fp16 on the scalar engine.
  - Softmax denominator, split to balance engines: even t-blocks are
    column-summed on the PE with an M=128 all-ones stationary (PSUM
    accumulates a pre-broadcast [128,512] sum -- no separate broadcast
    matmul), odd t-blocks accumulate on DVE in fp16; one merge matmul
    folds the DVE chain into the same PSUM bank.  Reciprocal runs on
    the full [128,512] tile (a [1,512] DVE op costs 3.3us; avoid).
"""

import math
import sys

import numpy as np

for _p in ("/opt/trn_rl_repo",):
    if _p not in sys.path:
        sys.path.append(_p)

import bass_rust

import concourse.bass as bass
import concourse.tile as tile
from concourse import mybir
from concourse.bass_utils import run_bass_kernel_spmd
from concourse.masks import make_identity
from concourse.vector_clock import ScopedClock

S, B, D = 2048, 2, 2048
H, C = 16, 128
HL = 4                 # heads per core
M = HL * C             # local qkv rows (512)
EPS = 1e-6
NCORES = 8
ST = S // 128          # 16 s-blocks
DT = D // 128          # 16 d-blocks
NSC = S // 512         # 4 s-chunks for attention
SQRT_C = math.sqrt(C)

f32 = mybir.dt.float32
f16 = mybir.dt.float16
Act = mybir.ActivationFunctionType
Alu = mybir.AluOpType


# ---------------------------------------------------------------------------
# This container's walrus accepts at most one sync-wait command per
# instruction; the stock TileContext exit drain carries one wait per
# outstanding proc.  Split them onto single-wait NoOps.
def _split_drain_and_barrier(self, tick_clock, wait_clock):
    nc = self.nc
    probe = nc.sync.nop(nofuse=True, hint="tile_exit_waits")
    wait_clock.add_sem_waits(probe.ins, ScopedClock({None: tick_clock.global_clock}))
    si = probe.ins.sync_info
    if si is not None and si.on_wait is not None and len(si.on_wait) > 1:
        waits = list(si.on_wait)
        si.on_wait = [waits[0]]
        for w in waits[1:]:
            n2 = nc.sync.nop(nofuse=True, hint="tile_exit_waits")
            n2.ins.sync_info = bass_rust.SyncInfo(on_wait=[w], on_update=[])
    nc.sync.drain(fusable=False)
    nc.all_engine_barrier()
    popped = nc._tile_sem_poison_stack.pop()
    assert popped is self._sem_poison
    nc.clear_and_free_semaphores(list(self.sems.allocated().values()))
    nc.all_engine_barrier()


tile.TileContext._drain_and_barrier = _split_drain_and_barrier


def _split_multi_waits(nc):
    """Walrus here accepts one sync-wait per instruction; hoist extras onto
    single-wait NoOps on the same engine immediately before the instruction."""
    for f in nc.m.functions:
        for bb in f.blocks:
            out = []
            changed = False
            for inst in bb.instructions:
                si = inst.sync_info
                if si is not None and si.on_wait is not None and len(si.on_wait) > 1:
                    waits = list(si.on_wait)
                    si.on_wait = [waits[-1]]
                    for w in waits[:-1]:
                        nop = mybir.InstNoOp(
                            name=f"I-{nc.next_id()}",
                            engine=inst.engine,
                            sync_info=mybir.SyncInfo(on_wait=[w], on_update=[]),
                            bass_nofuse=True,
                        )
                        out.append(nop)
                    changed = True
                out.append(inst)
            if changed:
                bb.instructions[:] = out


def _bcast_heads(ap_2d, heads):
    """View a [128, C] AP as [128, heads, C] with a zero-stride head dim."""
    return bass.AP(
        tensor=ap_2d.tensor,
        offset=ap_2d.offset,
        ap=[ap_2d.ap[0], [0, heads], ap_2d.ap[1]],
    )


def build_core_kernel(split_waits=True, finish_ln=False):
    """One core's kernel: partial attention output for 4 heads of 1 batch."""
    nc = bass.Bass()

    # host-prearranged layouts (see make_in_maps):
    #   xp[p, (j, n, c)]  = x[j*128+c_s, n*128+p]   (x^T tiles per s-block)
    #   wq[p, (n, m)]     = Wq_loc[m, n*128+p]
    #   wout[p, (h, e)]   = Wout_loc[e, h*128+p]
    #   cosf/ssinf[p, (j, c)] = table[j*128+p, c]
    xp = nc.declare_dram_parameter("xp", [128, ST * DT * 128], f16, isOutput=False)
    wq = nc.declare_dram_parameter("wq", [128, DT * M], f16, isOutput=False)
    wk = nc.declare_dram_parameter("wk", [128, DT * M], f16, isOutput=False)
    wv = nc.declare_dram_parameter("wv", [128, DT * M], f16, isOutput=False)
    wout = nc.declare_dram_parameter("wout", [128, HL * D], f16, isOutput=False)
    cosf = nc.declare_dram_parameter("cosf", [128, ST * C], f16, isOutput=False)
    ssinf = nc.declare_dram_parameter("ssinf", [128, ST * C], f16, isOutput=False)
    qs = nc.declare_dram_parameter("qs", [C], f16, isOutput=False)
    ks = nc.declare_dram_parameter("ks", [C], f16, isOutput=False)
    out = nc.declare_dram_parameter("out", [S, D], f16, isOutput=True)

    xp_r = xp.rearrange("p (j n c) -> p j n c", j=ST, n=DT)
    wq_r = wq.rearrange("p (n m) -> p n m", n=DT)
    wk_r = wk.rearrange("p (n m) -> p n m", n=DT)
    wv_r = wv.rearrange("p (n m) -> p n m", n=DT)
    wout_r = wout.rearrange("p (h e) -> p h e", h=HL)
    cos_r = cosf.rearrange("p (j c) -> p j c", j=ST)
    ssin_r = ssinf.rearrange("p (j c) -> p j c", j=ST)

    with tile.TileContext(nc) as tc:
        with (
            tc.tile_pool(name="const", bufs=1) as constp,
            tc.tile_pool(name="qkt", bufs=1) as qktp,
            tc.tile_pool(name="vpool", bufs=1) as vpool,
            tc.tile_pool(name="woutp", bufs=1) as woutp,
        ):
            # ---- constants ----
            ident = constp.tile([128, 128], f16, name="ident")
            make_identity(nc, ident)
            ones16 = constp.tile([128, 128], f16, name="ones16")
            nc.vector.memset(ones16, 1.0)
            # bias for rms' = sqrt(ssq/sqrt(C) + eps*sqrt(C)) = rms*C^0.25
            epsb = constp.tile([128, 1], f32, name="epsb")
            nc.vector.memset(epsb, EPS * SQRT_C)

            # qs/ks scale tiles: allocated here, DMA'd after the first
            # weight group so weight bytes hit the DMA pipe first
            qs_bc = constp.tile([128, C], f16, name="qs_bc")
            ks_bc = constp.tile([128, C], f16, name="ks_bc")
            qs_rot = constp.tile([128, C], f16, name="qs_rot")
            ks_rot = constp.tile([128, C], f16, name="ks_rot")

            qT = qktp.tile([128, HL, S], f16, name="qT")
            kT = qktp.tile([128, HL, S], f16, name="kT")
            v_sb = vpool.tile([128, ST, M], f16, name="v_sb")
            wout_sb = woutp.tile([128, HL, D], f16, name="wout_sb")

            # ---- phase 1: QKV projection + rmsnorm + rope + transpose ----
            with (
                tc.tile_pool(name="wqkv", bufs=1) as wqkvp,
                tc.tile_pool(name="rope", bufs=1) as ropep,
                tc.tile_pool(name="ph1", bufs=2) as ph1,
                tc.tile_pool(name="accps", bufs=6, space="PSUM") as accps,
                tc.tile_pool(name="tps", bufs=2, space="PSUM") as tps,
            ):
                wq_sb = wqkvp.tile([128, DT, M], f16, name="wq_sb")
                wk_sb = wqkvp.tile([128, DT, M], f16, name="wk_sb")
                wv_sb = wqkvp.tile([128, DT, M], f16, name="wv_sb")
                # DMA schedule: small first n-group so the first matmul can
                # start early; x tiles and rope tables interleaved; wout and
                # remaining x tiles stream during the j loop.
                WGRPS = [(0, 1), (1, 3), (3, 6), (6, 10), (10, 16)]
                for w_sb, w_r in ((wq_sb, wq_r), (wk_sb, wk_r), (wv_sb, wv_r)):
                    lo, hi = WGRPS[0]
                    nc.sync.dma_start(out=w_sb[:, lo:hi, :], in_=w_r[:, lo:hi, :])
                xjs = {}
                for j in (0, 1):
                    xj = ph1.tile(
                        [128, DT, 128], f16, name="xj", tag="xj", bufs=6
                    )
                    if j == 0:
                        # split so the first matmul gates on n=0..3 only
                        nc.sync.dma_start(
                            out=xj[:, 0:4, :], in_=xp_r[:, j, 0:4, :]
                        )
                        nc.sync.dma_start(
                            out=xj[:, 4:DT, :], in_=xp_r[:, j, 4:DT, :]
                        )
                    else:
                        nc.sync.dma_start(out=xj, in_=xp_r[:, j, :, :])
                    xjs[j] = xj
                for w_bc, w_dram in ((qs_bc, qs), (ks_bc, ks)):
                    src = bass.AP(
                        tensor=w_dram.ap().tensor, offset=0, ap=[[0, 128], [1, C]]
                    )
                    nc.sync.dma_start(out=w_bc, in_=src)
                for w_rot, w_bc in ((qs_rot, qs_bc), (ks_rot, ks_bc)):
                    nc.gpsimd.tensor_copy(
                        out=w_rot[:, 0 : C // 2], in_=w_bc[:, C // 2 : C]
                    )
                    nc.gpsimd.tensor_copy(
                        out=w_rot[:, C // 2 : C], in_=w_bc[:, 0 : C // 2]
                    )
                # PE warmup on resident constants while the weight DMAs
                # stream: ~5us of dummy matmuls gets HAM to 8/8 before the
                # first real matmul instead of paying the cold clock on it
                warm = accps.tile([128, 512], f32, name="warm", tag="acc")
                for _ in range(72):
                    nc.tensor.matmul(
                        warm[:, 0:128], lhsT=ident, rhs=ones16,
                        start=True, stop=True,
                    )
                cos_t = ropep.tile([128, ST, C], f16, name="cos_t")
                ssin_t = ropep.tile([128, ST, C], f16, name="ssin_t")
                for gi, (lo, hi) in enumerate(WGRPS[1:]):
                    for w_sb, w_r in ((wq_sb, wq_r), (wk_sb, wk_r), (wv_sb, wv_r)):
                        nc.sync.dma_start(
                            out=w_sb[:, lo:hi, :], in_=w_r[:, lo:hi, :]
                        )
                    if gi == 0:
                        nc.sync.dma_start(out=cos_t, in_=cos_r)
                        nc.sync.dma_start(out=ssin_t, in_=ssin_r)

                pend_tr = []  # deferred transposes: (t1, dstT, j)

                def flush_transposes(upto_j=None):
                    while pend_tr and (
                        upto_j is None or pend_tr[0][2] <= upto_j
                    ):
                        t1, dstT, j = pend_tr.pop(0)
                        pt = tps.tile([128, M], f16, name="pt")
                        for h in range(HL):
                            nc.tensor.transpose(
                                pt[:, h * C : (h + 1) * C], t1[:, h, :], ident
                            )
                        nc.vector.tensor_copy(
                            out=dstT[:, :, j * 128 : (j + 1) * 128],
                            in_=pt.rearrange("p (a c) -> p a c", a=HL),
                        )

                for j in range(ST):
                    if j in xjs:
                        xj = xjs.pop(j)
                    else:
                        xj = ph1.tile(
                            [128, DT, 128], f16, name="xj", tag="xj", bufs=6
                        )
                        nc.sync.dma_start(out=xj, in_=xp_r[:, j, :, :])
                    if j == 3:
                        for h in range(HL):
                            nc.sync.dma_start(
                                out=wout_sb[:, h, :], in_=wout_r[:, h, :]
                            )
                    # rope tables for this j on GpSimd, ahead of the chain
                    ropes = {}
                    for key, tab, w_bc, w_rot in (
                        ("q", cos_t, qs_bc, qs_rot),
                        ("k", cos_t, ks_bc, ks_rot),
                    ):
                        cwF = ph1.tile([128, M], f16, name="cwF", bufs=4, tag="cwF")
                        nc.gpsimd.tensor_mul(
                            out=cwF.rearrange("p (a c) -> p a c", a=HL),
                            in0=_bcast_heads(cos_t[:, j, :], HL),
                            in1=_bcast_heads(w_bc, HL),
                        )
                        swF = ph1.tile([128, M], f16, name="swF", bufs=4, tag="swF")
                        nc.gpsimd.tensor_mul(
                            out=swF.rearrange("p (a c) -> p a c", a=HL),
                            in0=_bcast_heads(ssin_t[:, j, :], HL),
                            in1=_bcast_heads(w_rot, HL),
                        )
                        ropes[key] = (cwF, swF)
                    pq = accps.tile([128, M], f32, name="pq", tag="acc")
                    pk = accps.tile([128, M], f32, name="pk", tag="acc")
                    pv = accps.tile([128, M], f32, name="pv", tag="acc")
                    for n in range(DT):
                        if n == 12:
                            flush_transposes(upto_j=j - 2)
                        xsl = xj[:, n, :]
                        nc.tensor.matmul(
                            pq, lhsT=xsl, rhs=wq_sb[:, n, :],
                            start=(n == 0), stop=(n == DT - 1),
                        )
                        nc.tensor.matmul(
                            pk, lhsT=xsl, rhs=wk_sb[:, n, :],
                            start=(n == 0), stop=(n == DT - 1),
                        )
                        nc.tensor.matmul(
                            pv, lhsT=xsl, rhs=wv_sb[:, n, :],
                            start=(n == 0), stop=(n == DT - 1),
                        )
                    # v: plain copy to SBUF (cast fp16)
                    nc.scalar.copy(out=v_sb[:, j, :], in_=pv)
                    # q, k: rmsnorm scale + rope, all heads at once
                    for pacc, key, dstT in (
                        (pq, "q", qT),
                        (pk, "k", kT),
                    ):
                        cwF, swF = ropes[key]
                        xq = ph1.tile([128, M], f16, name="xq", bufs=4)
                        nc.scalar.copy(out=xq, in_=pacc)
                        sq = ph1.tile([128, M], f16, name="sq", bufs=2)
                        nc.gpsimd.tensor_mul(out=sq, in0=xq, in1=xq)
                        ssq4 = ph1.tile([128, HL, 1], f32, name="ssq4", bufs=3)
                        nc.vector.tensor_reduce(
                            out=ssq4,
                            in_=sq.rearrange("p (a c) -> p a c", a=HL),
                            op=Alu.add,
                            axis=mybir.AxisListType.X,
                        )
                        rms4 = ph1.tile([128, HL], f32, name="rms4", bufs=3)
                        nc.scalar.activation(
                            out=rms4,
                            in_=ssq4.rearrange("p a one -> p (a one)"),
                            func=Act.Sqrt, scale=1.0 / SQRT_C, bias=epsb,
                        )
                        r4 = ph1.tile([128, HL], f32, name="r4", bufs=3)
                        nc.vector.reciprocal(out=r4, in_=rms4)
                        # per-head 1/(rms*C^0.25) on DVE (keeps chain local)
                        xqs = ph1.tile([128, M], f16, name="xqs", bufs=3)
                        for h in range(HL):
                            nc.vector.tensor_scalar(
                                out=xqs[:, h * C : (h + 1) * C],
                                in0=xq[:, h * C : (h + 1) * C],
                                scalar1=r4[:, h : h + 1],
                                scalar2=None,
                                op0=Alu.mult,
                            )
                        # rotate_half
                        xqs3 = xqs.rearrange("p (a c) -> p a c", a=HL)
                        sh = ph1.tile([128, HL, C], f16, name="sh", bufs=3)
                        nc.vector.tensor_copy(
                            out=sh[:, :, 0 : C // 2], in_=xqs3[:, :, C // 2 : C]
                        )
                        nc.vector.tensor_copy(
                            out=sh[:, :, C // 2 : C], in_=xqs3[:, :, 0 : C // 2]
                        )
                        t1 = ph1.tile([128, HL, C], f16, name="t1", bufs=6)
                        t1f = t1.rearrange("p a c -> p (a c)")
                        shf = sh.rearrange("p a c -> p (a c)")
                        nc.vector.tensor_mul(out=t1f, in0=xqs, in1=cwF)
                        nc.vector.tensor_mul(out=shf, in0=shf, in1=swF)
                        nc.vector.tensor_add(out=t1f, in0=t1f, in1=shf)
                        pend_tr.append((t1, dstT, j))
                    if j == ST - 1:
                        # preload the attention table set while the PE is
                        # still busy with the last block's matmuls
                        dummy = ph1.tile([1, 1], f16, name="dummy", bufs=1)
                        nc.scalar.activation(
                            out=dummy, in_=epsb[0:1, :],
                            func=Act.Ln if finish_ln else Act.Exp,
                        )
                flush_transposes()

            # ---- phase 2: attention + output projection ----
            with (
                tc.tile_pool(name="att", bufs=3) as attp,
                tc.tile_pool(name="outT", bufs=2) as outTp,
                tc.tile_pool(name="scps", bufs=2, space="PSUM") as scps,
                tc.tile_pool(name="ops", bufs=2, space="PSUM") as ops,
                tc.tile_pool(name="dbp", bufs=2, space="PSUM") as dbp,
            ):
                def finish_head(fin):
                    """Merge the DVE exp-sum chain into the PE's pre-broadcast
                    PSUM denominator, then reciprocal + divide on DVE."""
                    po, den_bc, esumA, outT_slice = fin
                    nc.tensor.matmul(
                        den_bc, lhsT=ones16, rhs=esumA,
                        start=False, stop=True,
                    )
                    rinv = attp.tile([128, 512], f16, name="rinv", bufs=2)
                    if finish_ln:
                        # 1/den = exp(-ln(den)) on ACT (2x735ns; DVE exact
                        # reciprocal costs 3.35us at 6.5 cyc/elem)
                        lden = attp.tile([128, 512], f16, name="lden", bufs=2)
                        nc.scalar.activation(out=lden, in_=den_bc, func=Act.Ln)
                        nc.scalar.activation(
                            out=rinv, in_=lden, func=Act.Exp, scale=-1.0
                        )
                    else:
                        with nc.allow_low_precision(
                            reason="1/denominator fp16: 5e-4 rel on 2e-2 budget"
                        ):
                            nc.vector.reciprocal(out=rinv, in_=den_bc)
                    nc.vector.tensor_mul(out=outT_slice, in0=po, in1=rinv)

                pending = []
                for nchunk in range(NSC):
                    ssl = slice(nchunk * 512, (nchunk + 1) * 512)
                    outT_n = outTp.tile([128, HL, 512], f16, name="outT_n")
                    for h in range(HL):
                        po = ops.tile([128, 512], f32, name="po", tag="o")
                        den_bc = dbp.tile([128, 512], f32, name="den_bc", tag="d")
                        esumA = attp.tile([128, 512], f16, name="esA", bufs=2)
                        for tp in range(ST // 2):
                            t0, t1b = 2 * tp, 2 * tp + 1
                            psc = scps.tile([128, 1024], f32, name="psc", tag="sc")
                            nc.tensor.matmul(
                                psc[:, 0:512],
                                lhsT=kT[:, h, t0 * 128 : (t0 + 1) * 128],
                                rhs=qT[:, h, ssl],
                                start=True, stop=True,
                            )
                            nc.tensor.matmul(
                                psc[:, 512:1024],
                                lhsT=kT[:, h, t1b * 128 : (t1b + 1) * 128],
                                rhs=qT[:, h, ssl],
                                start=True, stop=True,
                            )
                            if tp == 1 and pending:
                                finish_head(pending.pop())
                            e2 = attp.tile([128, 1024], f16, name="e2", bufs=4)
                            nc.scalar.activation(out=e2, in_=psc, func=Act.Exp)
                            nc.tensor.matmul(
                                po,
                                lhsT=v_sb[:, t0, h * C : (h + 1) * C],
                                rhs=e2[:, 0:512],
                                start=(t0 == 0), stop=(t0 == ST - 1),
                            )
                            nc.tensor.matmul(
                                po,
                                lhsT=v_sb[:, t1b, h * C : (h + 1) * C],
                                rhs=e2[:, 512:1024],
                                start=(t1b == 0), stop=(t1b == ST - 1),
                            )
                            # denominator: even half on PE (pre-broadcast
                            # column sums), odd half on DVE fp16 chain
                            nc.tensor.matmul(
                                den_bc, lhsT=ones16, rhs=e2[:, 0:512],
                                start=(tp == 0), stop=False,
                            )
                            if tp == 0:
                                nc.vector.tensor_copy(
                                    out=esumA, in_=e2[:, 512:1024]
                                )
                            else:
                                nc.vector.tensor_add(
                                    out=esumA, in0=esumA, in1=e2[:, 512:1024]
                                )
                        pending.append((po, den_bc, esumA, outT_n[:, h, :]))
                    while pending:
                        finish_head(pending.pop())

                    def finish_proj(ent):
                        psum_out, jj, dc = ent
                        srow = (nchunk * 4 + jj) * 128
                        nc.tensor.matmul(
                            psum_out,
                            lhsT=outT_n[:, HL - 1, jj * 128 : (jj + 1) * 128],
                            rhs=wout_sb[:, HL - 1, dc * 512 : (dc + 1) * 512],
                            start=False, stop=True,
                        )
                        out_sb = attp.tile(
                            [128, 512], f16, name="out_sb", bufs=6
                        )
                        nc.vector.tensor_copy(out=out_sb, in_=psum_out)
                        nc.sync.dma_start(
                            out=out[srow : srow + 128, dc * 512 : (dc + 1) * 512],
                            in_=out_sb,
                        )

                    # h0-h2 accumulation runs ahead as PE filler while the
                    # last head's normalization chain (merge/recip/divide)
                    # completes; the h3 matmul joins per group afterwards.
                    # Even groups borrow the den banks (idle during the
                    # projection), odd groups the PV banks: 4 rings in flight.
                    proj_pend = []
                    for g in range(16):
                        jj, dc = g // 4, g % 4
                        if g % 2 == 0:
                            psum_out = dbp.tile(
                                [128, 512], f32, name="psum_out", tag="d"
                            )
                        else:
                            psum_out = ops.tile(
                                [128, 512], f32, name="psum_out", tag="o"
                            )
                        for h in range(HL - 1):
                            nc.tensor.matmul(
                                psum_out,
                                lhsT=outT_n[:, h, jj * 128 : (jj + 1) * 128],
                                rhs=wout_sb[:, h, dc * 512 : (dc + 1) * 512],
                                start=(h == 0), stop=False,
                            )
                        proj_pend.append((psum_out, jj, dc))
                        if g >= 3:
                            finish_proj(proj_pend.pop(0))
                    while proj_pend:
                        finish_proj(proj_pend.pop(0))
    if split_waits:
        _split_multi_waits(nc)
    return nc


_NC_CACHE = {}


def _get_nc():
    if "nc" not in _NC_CACHE:
        _NC_CACHE["nc"] = build_core_kernel()
    return _NC_CACHE["nc"]


def make_in_maps(x, rope_emb, Wq, Wk, Wv, Wout, q_scale, k_scale):
    freqs = rope_emb.reshape(S, C).astype(np.float64)
    cosf = np.cos(freqs)
    sf = np.sin(freqs)
    ssinf = np.concatenate([-sf[:, : C // 2], sf[:, C // 2 :]], axis=1)
    # [p, (j, c)] layout for the rope tables
    cos_p = np.ascontiguousarray(
        cosf.reshape(ST, 128, C).transpose(1, 0, 2).reshape(128, -1), np.float16
    )
    ssin_p = np.ascontiguousarray(
        ssinf.reshape(ST, 128, C).transpose(1, 0, 2).reshape(128, -1), np.float16
    )
    in_maps = []
    for c in range(NCORES):
        b, hg = c // 4, c % 4
        sl = slice(hg * M, (hg + 1) * M)
        x_b = np.asarray(x[:, b, :], np.float16)  # [S, D]
        # xp[p, j, n, sc] = x_b[j*128+sc, n*128+p]
        xp = np.ascontiguousarray(
            x_b.reshape(ST, 128, DT, 128).transpose(3, 0, 2, 1).reshape(128, -1)
        )
        def wlayout(Wl):  # Wl: [M, D] -> [p, (n, m)]
            return np.ascontiguousarray(
                Wl.T.reshape(DT, 128, M).transpose(1, 0, 2).reshape(128, -1),
                np.float16,
            )
        # wout[p, (h, e)] = Wout[:, sl][e, h*128+p]
        wout_l = np.ascontiguousarray(
            Wout[:, sl].T.reshape(HL, 128, D).transpose(1, 0, 2).reshape(128, -1),
            np.float16,
        )
        in_maps.append(
            {
                "xp": xp,
                "wq": wlayout(np.asarray(Wq[sl, :], np.float32)),
                "wk": wlayout(np.asarray(Wk[sl, :], np.float32)),
                "wv": wlayout(np.asarray(Wv[sl, :], np.float32)),
                "wout": wout_l,
                "cosf": cos_p,
                "ssinf": ssin_p,
                "qs": np.ascontiguousarray(q_scale, np.float16),
                "ks": np.ascontiguousarray(k_scale, np.float16),
            }
        )
    return in_maps


def kernel(x, rope_emb, Wq, Wk, Wv, Wout, q_scale, k_scale, **run_kwargs):
    in_maps = make_in_maps(
        np.asarray(x, np.float32),
        np.asarray(rope_emb, np.float32),
        np.asarray(Wq, np.float32),
        np.asarray(Wk, np.float32),
        np.asarray(Wv, np.float32),
        np.asarray(Wout, np.float32),
        np.asarray(q_scale, np.float32),
        np.asarray(k_scale, np.float32),
    )
    nc = _get_nc()
    res = run_bass_kernel_spmd(nc, in_maps, core_ids=list(range(NCORES)), **run_kwargs)
    out = np.zeros((S, B, D), dtype=np.float32)
    for c in range(NCORES):
        out[:, c // 4, :] += np.asarray(res.results[c]["out"], np.float32)
    if run_kwargs.get("trace"):
        kernel.last_result = res
    return out


# revision 36
# speedup vs baseline: 1.0262x; 1.0262x over previous
"""Distributed Trainium2 kernel for nn_Attention_31370441130243.

Full-input / full-output attention layer, sharded internally over the
8 NeuronCores as (batch=2) x (head-group=4): core c handles batch c//4
and heads [4*(c%4), 4*(c%4)+4).  Each core computes its QKV projections,
per-head RMSNorm + RoPE, non-causal SDPA and a partial output projection
(its Wout column block); the host sums the 4 partials per batch.

v3 design (all 16-bit tensors are fp16; PSUM accumulation is fp32):
  - One pass over x computes q, k, v per 128-row s-block (j): the three
    matmuls share each stationary x tile, so x streams from HBM once.
  - RMSNorm: 1/(rms*C^0.25) is applied to BOTH q and k (scalar-engine
    copy-with-per-partition-scale), so the score exp needs no scale.
  - RoPE tables are pre-broadened to [128, 512] on GpSimd so the DVE
    combine is three flat 1-D ops; PE transposes in fp16 are deferred
    into the next block's matmul stream to hide the elementwise chain.
  - Scores are computed transposed (scT = kT_blk.T @ qT = [t, s]) so the
    PV matmul needs no P transpose; exp -> fp16 on the scalar engine.
  - Softmax denominator, split to balance engines: even t-blocks are
    column-summed on the PE with an M=128 all-ones stationary (PSUM
    accumulates a pre-broadcast [128,512] sum -- no separate broadcast
    matmul), odd t-blocks accumulate on DVE in fp16; one merge matmul
    folds the DVE chain into the same PSUM bank.  Reciprocal runs on
    the full [128,512] tile (a [1,512] DVE op costs 3.3us; avoid).
"""

import math
import sys

import numpy as np

for _p in ("/opt/trn_rl_repo",):
    if _p not in sys.path:
        sys.path.append(_p)

import bass_rust

import concourse.bass as bass
import concourse.tile as tile
from concourse import mybir
from concourse.bass_utils import run_bass_kernel_spmd
from concourse.masks import make_identity
from concourse.vector_clock import ScopedClock

S, B, D = 2048, 2, 2048
H, C = 16, 128
HL = 4                 # heads per core
M = HL * C             # local qkv rows (512)
EPS = 1e-6
NCORES = 8
ST = S // 128          # 16 s-blocks
DT = D // 128          # 16 d-blocks
NSC = S // 512         # 4 s-chunks for attention
SQRT_C = math.sqrt(C)

f32 = mybir.dt.float32
f16 = mybir.dt.float16
Act = mybir.ActivationFunctionType
Alu = mybir.AluOpType


# ---------------------------------------------------------------------------
# This container's walrus accepts at most one sync-wait command per
# instruction; the stock TileContext exit drain carries one wait per
# outstanding proc.  Split them onto single-wait NoOps.
def _split_drain_and_barrier(self, tick_clock, wait_clock):
    nc = self.nc
    probe = nc.sync.nop(nofuse=True, hint="tile_exit_waits")
    wait_clock.add_sem_waits(probe.ins, ScopedClock({None: tick_clock.global_clock}))
    si = probe.ins.sync_info
    if si is not None and si.on_wait is not None and len(si.on_wait) > 1:
        waits = list(si.on_wait)
        si.on_wait = [waits[0]]
        for w in waits[1:]:
            n2 = nc.sync.nop(nofuse=True, hint="tile_exit_waits")
            n2.ins.sync_info = bass_rust.SyncInfo(on_wait=[w], on_update=[])
    nc.sync.drain(fusable=False)
    nc.all_engine_barrier()
    popped = nc._tile_sem_poison_stack.pop()
    assert popped is self._sem_poison
    nc.clear_and_free_semaphores(list(self.sems.allocated().values()))
    nc.all_engine_barrier()


tile.TileContext._drain_and_barrier = _split_drain_and_barrier


def _split_multi_waits(nc):
    """Walrus here accepts one sync-wait per instruction; hoist extras onto
    single-wait NoOps on the same engine immediately before the instruction."""
    for f in nc.m.functions:
        for bb in f.blocks:
            out = []
            changed = False
            for inst in bb.instructions:
                si = inst.sync_info
                if si is not None and si.on_wait is not None and len(si.on_wait) > 1:
                    waits = list(si.on_wait)
                    si.on_wait = [waits[-1]]
                    for w in waits[:-1]:
                        nop = mybir.InstNoOp(
                            name=f"I-{nc.next_id()}",
                            engine=inst.engine,
                            sync_info=mybir.SyncInfo(on_wait=[w], on_update=[]),
                            bass_nofuse=True,
                        )
                        out.append(nop)
                    changed = True
                out.append(inst)
            if changed:
                bb.instructions[:] = out


def _bcast_heads(ap_2d, heads):
    """View a [128, C] AP as [128, heads, C] with a zero-stride head dim."""
    return bass.AP(
        tensor=ap_2d.tensor,
        offset=ap_2d.offset,
        ap=[ap_2d.ap[0], [0, heads], ap_2d.ap[1]],
    )


def build_core_kernel(split_waits=True, finish_ln=False):
    """One core's kernel: partial attention output for 4 heads of 1 batch."""
    nc = bass.Bass()

    # host-prearranged layouts (see make_in_maps):
    #   xp[p, (j, n, c)]  = x[j*128+c_s, n*128+p]   (x^T tiles per s-block)
    #   wq[p, (n, m)]     = Wq_loc[m, n*128+p]
    #   wout[p, (h, e)]   = Wout_loc[e, h*128+p]
    #   cosf/ssinf[p, (j, c)] = table[j*128+p, c]
    xp = nc.declare_dram_parameter("xp", [128, ST * DT * 128], f16, isOutput=False)
    wq = nc.declare_dram_parameter("wq", [128, DT * M], f16, isOutput=False)
    wk = nc.declare_dram_parameter("wk", [128, DT * M], f16, isOutput=False)
    wv = nc.declare_dram_parameter("wv", [128, DT * M], f16, isOutput=False)
    wout = nc.declare_dram_parameter("wout", [128, HL * D], f16, isOutput=False)
    cosf = nc.declare_dram_parameter("cosf", [128, ST * C], f16, isOutput=False)
    ssinf = nc.declare_dram_parameter("ssinf", [128, ST * C], f16, isOutput=False)
    qs = nc.declare_dram_parameter("qs", [C], f16, isOutput=False)
    ks = nc.declare_dram_parameter("ks", [C], f16, isOutput=False)
    out = nc.declare_dram_parameter("out", [S, D], f16, isOutput=True)

    xp_r = xp.rearrange("p (j n c) -> p j n c", j=ST, n=DT)
    wq_r = wq.rearrange("p (n m) -> p n m", n=DT)
    wk_r = wk.rearrange("p (n m) -> p n m", n=DT)
    wv_r = wv.rearrange("p (n m) -> p n m", n=DT)
    wout_r = wout.rearrange("p (h e) -> p h e", h=HL)
    cos_r = cosf.rearrange("p (j c) -> p j c", j=ST)
    ssin_r = ssinf.rearrange("p (j c) -> p j c", j=ST)

    with tile.TileContext(nc) as tc:
        with (
            tc.tile_pool(name="const", bufs=1) as constp,
            tc.tile_pool(name="qkt", bufs=1) as qktp,
            tc.tile_pool(name="vpool", bufs=1) as vpool,
            tc.tile_pool(name="woutp", bufs=1) as woutp,
        ):
            # ---- constants ----
            ident = constp.tile([128, 128], f16, name="ident")
            make_identity(nc, ident)
            ones16 = constp.tile([128, 128], f16, name="ones16")
            nc.vector.memset(ones16, 1.0)
            # bias for rms' = sqrt(ssq/sqrt(C) + eps*sqrt(C)) = rms*C^0.25
            epsb = constp.tile([128, 1], f32, name="epsb")
            nc.vector.memset(epsb, EPS * SQRT_C)

            # qs/ks scale tiles: allocated here, DMA'd after the first
            # weight group so weight bytes hit the DMA pipe first
            qs_bc = constp.tile([128, C], f16, name="qs_bc")
            ks_bc = constp.tile([128, C], f16, name="ks_bc")
            qs_rot = constp.tile([128, C], f16, name="qs_rot")
            ks_rot = constp.tile([128, C], f16, name="ks_rot")

            qT = qktp.tile([128, HL, S], f16, name="qT")
            kT = qktp.tile([128, HL, S], f16, name="kT")
            v_sb = vpool.tile([128, ST, M], f16, name="v_sb")
            wout_sb = woutp.tile([128, HL, D], f16, name="wout_sb")

            # ---- phase 1: QKV projection + rmsnorm + rope + transpose ----
            with (
                tc.tile_pool(name="wqkv", bufs=1) as wqkvp,
                tc.tile_pool(name="rope", bufs=1) as ropep,
                tc.tile_pool(name="ph1", bufs=2) as ph1,
                tc.tile_pool(name="accps", bufs=6, space="PSUM") as accps,
                tc.tile_pool(name="tps", bufs=2, space="PSUM") as tps,
            ):
                wq_sb = wqkvp.tile([128, DT, M], f16, name="wq_sb")
                wk_sb = wqkvp.tile([128, DT, M], f16, name="wk_sb")
                wv_sb = wqkvp.tile([128, DT, M], f16, name="wv_sb")
                # DMA schedule: small first n-group so the first matmul can
                # start early; x tiles and rope tables interleaved; wout and
                # remaining x tiles stream during the j loop.
                WGRPS = [(0, 1), (1, 3), (3, 6), (6, 10), (10, 16)]
                for w_sb, w_r in ((wq_sb, wq_r), (wk_sb, wk_r), (wv_sb, wv_r)):
                    lo, hi = WGRPS[0]
                    nc.sync.dma_start(out=w_sb[:, lo:hi, :], in_=w_r[:, lo:hi, :])
                xjs = {}
                for j in (0, 1):
                    xj = ph1.tile(
                        [128, DT, 128], f16, name="xj", tag="xj", bufs=6
                    )
                    if j == 0:
                        # split so the first matmul gates on n=0..3 only
                        nc.sync.dma_start(
                            out=xj[:, 0:4, :], in_=xp_r[:, j, 0:4, :]
                        )
                        nc.sync.dma_start(
                            out=xj[:, 4:DT, :], in_=xp_r[:, j, 4:DT, :]
                        )
                    else:
                        nc.sync.dma_start(out=xj, in_=xp_r[:, j, :, :])
                    xjs[j] = xj
                for w_bc, w_dram in ((qs_bc, qs), (ks_bc, ks)):
                    src = bass.AP(
                        tensor=w_dram.ap().tensor, offset=0, ap=[[0, 128], [1, C]]
                    )
                    nc.sync.dma_start(out=w_bc, in_=src)
                for w_rot, w_bc in ((qs_rot, qs_bc), (ks_rot, ks_bc)):
                    nc.gpsimd.tensor_copy(
                        out=w_rot[:, 0 : C // 2], in_=w_bc[:, C // 2 : C]
                    )
                    nc.gpsimd.tensor_copy(
                        out=w_rot[:, C // 2 : C], in_=w_bc[:, 0 : C // 2]
                    )
                # PE warmup on resident constants while the weight DMAs
                # stream: ~5us of dummy matmuls gets HAM to 8/8 before the
                # first real matmul instead of paying the cold clock on it
                warm = accps.tile([128, 512], f32, name="warm", tag="acc")
                for _ in range(72):
                    nc.tensor.matmul(
                        warm[:, 0:128], lhsT=ident, rhs=ones16,
                        start=True, stop=True,
                    )
                cos_t = ropep.tile([128, ST, C], f16, name="cos_t")
                ssin_t = ropep.tile([128, ST, C], f16, name="ssin_t")
                for gi, (lo, hi) in enumerate(WGRPS[1:]):
                    for w_sb, w_r in ((wq_sb, wq_r), (wk_sb, wk_r), (wv_sb, wv_r)):
                        nc.sync.dma_start(
                            out=w_sb[:, lo:hi, :], in_=w_r[:, lo:hi, :]
                        )
                    if gi == 0:
                        nc.sync.dma_start(out=cos_t, in_=cos_r)
                        nc.sync.dma_start(out=ssin_t, in_=ssin_r)

                pend_tr = []  # deferred transposes: (t1, dstT, j)

                def flush_transposes(upto_j=None):
                    while pend_tr and (
                        upto_j is None or pend_tr[0][2] <= upto_j
                    ):
                        t1, dstT, j = pend_tr.pop(0)
                        pt = tps.tile([128, M], f16, name="pt")
                        for h in range(HL):
                            nc.tensor.transpose(
                                pt[:, h * C : (h + 1) * C], t1[:, h, :], ident
                            )
                        nc.vector.tensor_copy(
                            out=dstT[:, :, j * 128 : (j + 1) * 128],
                            in_=pt.rearrange("p (a c) -> p a c", a=HL),
                        )

                for j in range(ST):
                    if j in xjs:
                        xj = xjs.pop(j)
                    else:
                        xj = ph1.tile(
                            [128, DT, 128], f16, name="xj", tag="xj", bufs=6
                        )
                        nc.sync.dma_start(out=xj, in_=xp_r[:, j, :, :])
                    if j == 3:
                        for h in range(HL):
                            nc.sync.dma_start(
                                out=wout_sb[:, h, :], in_=wout_r[:, h, :]
                            )
                    # rope tables for this j on GpSimd, ahead of the chain
                    ropes = {}
                    for key, tab, w_bc, w_rot in (
                        ("q", cos_t, qs_bc, qs_rot),
                        ("k", cos_t, ks_bc, ks_rot),
                    ):
                        cwF = ph1.tile([128, M], f16, name="cwF", bufs=4, tag="cwF")
                        nc.gpsimd.tensor_mul(
                            out=cwF.rearrange("p (a c) -> p a c", a=HL),
                            in0=_bcast_heads(cos_t[:, j, :], HL),
                            in1=_bcast_heads(w_bc, HL),
                        )
                        swF = ph1.tile([128, M], f16, name="swF", bufs=4, tag="swF")
                        nc.gpsimd.tensor_mul(
                            out=swF.rearrange("p (a c) -> p a c", a=HL),
                            in0=_bcast_heads(ssin_t[:, j, :], HL),
                            in1=_bcast_heads(w_rot, HL),
                        )
                        ropes[key] = (cwF, swF)
                    pq = accps.tile([128, M], f32, name="pq", tag="acc")
                    pk = accps.tile([128, M], f32, name="pk", tag="acc")
                    pv = accps.tile([128, M], f32, name="pv", tag="acc")
                    for n in range(DT):
                        if n == 12:
                            flush_transposes(upto_j=j - 2)
                        xsl = xj[:, n, :]
                        nc.tensor.matmul(
                            pq, lhsT=xsl, rhs=wq_sb[:, n, :],
                            start=(n == 0), stop=(n == DT - 1),
                        )
                        nc.tensor.matmul(
                            pk, lhsT=xsl, rhs=wk_sb[:, n, :],
                            start=(n == 0), stop=(n == DT - 1),
                        )
                        nc.tensor.matmul(
                            pv, lhsT=xsl, rhs=wv_sb[:, n, :],
                            start=(n == 0), stop=(n == DT - 1),
                        )
                    # v: plain copy to SBUF (cast fp16)
                    nc.scalar.copy(out=v_sb[:, j, :], in_=pv)
                    # q, k: rmsnorm scale + rope, all heads at once
                    for pacc, key, dstT in (
                        (pq, "q", qT),
                        (pk, "k", kT),
                    ):
                        cwF, swF = ropes[key]
                        xq = ph1.tile([128, M], f16, name="xq", bufs=4)
                        nc.scalar.copy(out=xq, in_=pacc)
                        sq = ph1.tile([128, M], f16, name="sq", bufs=2)
                        nc.gpsimd.tensor_mul(out=sq, in0=xq, in1=xq)
                        ssq4 = ph1.tile([128, HL, 1], f32, name="ssq4", bufs=3)
                        nc.vector.tensor_reduce(
                            out=ssq4,
                            in_=sq.rearrange("p (a c) -> p a c", a=HL),
                            op=Alu.add,
                            axis=mybir.AxisListType.X,
                        )
                        rms4 = ph1.tile([128, HL], f32, name="rms4", bufs=3)
                        nc.scalar.activation(
                            out=rms4,
                            in_=ssq4.rearrange("p a one -> p (a one)"),
                            func=Act.Sqrt, scale=1.0 / SQRT_C, bias=epsb,
                        )
                        r4 = ph1.tile([128, HL], f32, name="r4", bufs=3)
                        nc.vector.reciprocal(out=r4, in_=rms4)
                        # per-head 1/(rms*C^0.25) on DVE (keeps chain local)
                        xqs = ph1.tile([128, M], f16, name="xqs", bufs=3)
                        for h in range(HL):
                            nc.vector.tensor_scalar(
                                out=xqs[:, h * C : (h + 1) * C],
                                in0=xq[:, h * C : (h + 1) * C],
                                scalar1=r4[:, h : h + 1],
                                scalar2=None,
                                op0=Alu.mult,
                            )
                        # rotate_half
                        xqs3 = xqs.rearrange("p (a c) -> p a c", a=HL)
                        sh = ph1.tile([128, HL, C], f16, name="sh", bufs=3)
                        nc.vector.tensor_copy(
                            out=sh[:, :, 0 : C // 2], in_=xqs3[:, :, C // 2 : C]
                        )
                        nc.vector.tensor_copy(
                            out=sh[:, :, C // 2 : C], in_=xqs3[:, :, 0 : C // 2]
                        )
                        t1 = ph1.tile([128, HL, C], f16, name="t1", bufs=6)
                        t1f = t1.rearrange("p a c -> p (a c)")
                        shf = sh.rearrange("p a c -> p (a c)")
                        nc.vector.tensor_mul(out=t1f, in0=xqs, in1=cwF)
                        nc.vector.tensor_mul(out=shf, in0=shf, in1=swF)
                        nc.vector.tensor_add(out=t1f, in0=t1f, in1=shf)
                        pend_tr.append((t1, dstT, j))
                    if j == ST - 1:
                        # preload the attention table set while the PE is
                        # still busy with the last block's matmuls
                        dummy = ph1.tile([1, 1], f16, name="dummy", bufs=1)
                        nc.scalar.activation(
                            out=dummy, in_=epsb[0:1, :],
                            func=Act.Ln,
                        )
                flush_transposes()

            # ---- phase 2: attention + output projection ----
            with (
                tc.tile_pool(name="att", bufs=3) as attp,
                tc.tile_pool(name="outT", bufs=2) as outTp,
                tc.tile_pool(name="scps", bufs=2, space="PSUM") as scps,
                tc.tile_pool(name="ops", bufs=2, space="PSUM") as ops,
                tc.tile_pool(name="dbp", bufs=2, space="PSUM") as dbp,
            ):
                def finish_head(fin, fast=False):
                    """Merge the DVE exp-sum chain into the PE's pre-broadcast
                    PSUM denominator, then reciprocal + divide on DVE."""
                    po, den_bc, esumA, outT_slice = fin
                    nc.tensor.matmul(
                        den_bc, lhsT=ones16, rhs=esumA,
                        start=False, stop=True,
                    )
                    rinv = attp.tile([128, 512], f16, name="rinv", bufs=2)
                    if fast or finish_ln:
                        # drained (last-per-chunk) head: exp(-ln(den)) on ACT
                        # (2x735ns) -- the 3.35us DVE reciprocal would sit on
                        # the critical chain gating den release + outT[h3]
                        lden = attp.tile([128, 512], f16, name="lden", bufs=2)
                        nc.scalar.activation(out=lden, in_=den_bc, func=Act.Ln)
                        nc.scalar.activation(
                            out=rinv, in_=lden, func=Act.Exp, scale=-1.0
                        )
                    else:
                        with nc.allow_low_precision(
                            reason="1/denominator fp16: 5e-4 rel on 2e-2 budget"
                        ):
                            nc.vector.reciprocal(out=rinv, in_=den_bc)
                    nc.vector.tensor_mul(out=outT_slice, in0=po, in1=rinv)

                pending = []
                for nchunk in range(NSC):
                    ssl = slice(nchunk * 512, (nchunk + 1) * 512)
                    outT_n = outTp.tile([128, HL, 512], f16, name="outT_n")
                    for h in range(HL):
                        po = ops.tile([128, 512], f32, name="po", tag="o")
                        den_bc = dbp.tile([128, 512], f32, name="den_bc", tag="d")
                        esumA = attp.tile([128, 512], f16, name="esA", bufs=2)
                        for tp in range(ST // 2):
                            t0, t1b = 2 * tp, 2 * tp + 1
                            psc = scps.tile([128, 1024], f32, name="psc", tag="sc")
                            nc.tensor.matmul(
                                psc[:, 0:512],
                                lhsT=kT[:, h, t0 * 128 : (t0 + 1) * 128],
                                rhs=qT[:, h, ssl],
                                start=True, stop=True,
                            )
                            nc.tensor.matmul(
                                psc[:, 512:1024],
                                lhsT=kT[:, h, t1b * 128 : (t1b + 1) * 128],
                                rhs=qT[:, h, ssl],
                                start=True, stop=True,
                            )
                            if tp == 1 and pending:
                                finish_head(pending.pop())
                            e2 = attp.tile([128, 1024], f16, name="e2", bufs=4)
                            nc.scalar.activation(out=e2, in_=psc, func=Act.Exp)
                            nc.tensor.matmul(
                                po,
                                lhsT=v_sb[:, t0, h * C : (h + 1) * C],
                                rhs=e2[:, 0:512],
                                start=(t0 == 0), stop=(t0 == ST - 1),
                            )
                            nc.tensor.matmul(
                                po,
                                lhsT=v_sb[:, t1b, h * C : (h + 1) * C],
                                rhs=e2[:, 512:1024],
                                start=(t1b == 0), stop=(t1b == ST - 1),
                            )
                            # denominator: even half on PE (pre-broadcast
                            # column sums), odd half on DVE fp16 chain
                            nc.tensor.matmul(
                                den_bc, lhsT=ones16, rhs=e2[:, 0:512],
                                start=(tp == 0), stop=False,
                            )
                            if tp == 0:
                                nc.vector.tensor_copy(
                                    out=esumA, in_=e2[:, 512:1024]
                                )
                            else:
                                nc.vector.tensor_add(
                                    out=esumA, in0=esumA, in1=e2[:, 512:1024]
                                )
                        pending.append((po, den_bc, esumA, outT_n[:, h, :]))
                    while pending:
                        finish_head(pending.pop(), fast=True)

                    def finish_proj(ent):
                        psum_out, jj, dc = ent
                        srow = (nchunk * 4 + jj) * 128
                        nc.tensor.matmul(
                            psum_out,
                            lhsT=outT_n[:, HL - 1, jj * 128 : (jj + 1) * 128],
                            rhs=wout_sb[:, HL - 1, dc * 512 : (dc + 1) * 512],
                            start=False, stop=True,
                        )
                        out_sb = attp.tile(
                            [128, 512], f16, name="out_sb", bufs=6
                        )
                        # alternate cast engine: a single DVE cast chain lags
                        # the matmuls and stalls the next chunk's first PV on
                        # the ring release
                        if (jj * 4 + dc) % 2 == 0:
                            nc.scalar.copy(out=out_sb, in_=psum_out)
                        else:
                            nc.vector.tensor_copy(out=out_sb, in_=psum_out)
                        nc.sync.dma_start(
                            out=out[srow : srow + 128, dc * 512 : (dc + 1) * 512],
                            in_=out_sb,
                        )

                    # h0-h2 accumulation runs ahead as PE filler while the
                    # last head's normalization chain (merge/recip/divide)
                    # completes; the h3 matmul joins per group afterwards.
                    # Even groups borrow the den banks (idle during the
                    # projection), odd groups the PV banks: 4 rings in flight.
                    proj_pend = []
                    for g in range(16):
                        jj, dc = g // 4, g % 4
                        if g % 2 == 0:
                            psum_out = dbp.tile(
                                [128, 512], f32, name="psum_out", tag="d"
                            )
                        else:
                            psum_out = ops.tile(
                                [128, 512], f32, name="psum_out", tag="o"
                            )
                        for h in range(HL - 1):
                            nc.tensor.matmul(
                                psum_out,
                                lhsT=outT_n[:, h, jj * 128 : (jj + 1) * 128],
                                rhs=wout_sb[:, h, dc * 512 : (dc + 1) * 512],
                                start=(h == 0), stop=False,
                            )
                        proj_pend.append((psum_out, jj, dc))
                        if g >= 3:
                            finish_proj(proj_pend.pop(0))
                    while proj_pend:
                        finish_proj(proj_pend.pop(0))
    if split_waits:
        _split_multi_waits(nc)
    return nc


_NC_CACHE = {}


def _get_nc():
    if "nc" not in _NC_CACHE:
        _NC_CACHE["nc"] = build_core_kernel()
    return _NC_CACHE["nc"]


def make_in_maps(x, rope_emb, Wq, Wk, Wv, Wout, q_scale, k_scale):
    freqs = rope_emb.reshape(S, C).astype(np.float64)
    cosf = np.cos(freqs)
    sf = np.sin(freqs)
    ssinf = np.concatenate([-sf[:, : C // 2], sf[:, C // 2 :]], axis=1)
    # [p, (j, c)] layout for the rope tables
    cos_p = np.ascontiguousarray(
        cosf.reshape(ST, 128, C).transpose(1, 0, 2).reshape(128, -1), np.float16
    )
    ssin_p = np.ascontiguousarray(
        ssinf.reshape(ST, 128, C).transpose(1, 0, 2).reshape(128, -1), np.float16
    )
    in_maps = []
    for c in range(NCORES):
        b, hg = c // 4, c % 4
        sl = slice(hg * M, (hg + 1) * M)
        x_b = np.asarray(x[:, b, :], np.float16)  # [S, D]
        # xp[p, j, n, sc] = x_b[j*128+sc, n*128+p]
        xp = np.ascontiguousarray(
            x_b.reshape(ST, 128, DT, 128).transpose(3, 0, 2, 1).reshape(128, -1)
        )
        def wlayout(Wl):  # Wl: [M, D] -> [p, (n, m)]
            return np.ascontiguousarray(
                Wl.T.reshape(DT, 128, M).transpose(1, 0, 2).reshape(128, -1),
                np.float16,
            )
        # wout[p, (h, e)] = Wout[:, sl][e, h*128+p]
        wout_l = np.ascontiguousarray(
            Wout[:, sl].T.reshape(HL, 128, D).transpose(1, 0, 2).reshape(128, -1),
            np.float16,
        )
        in_maps.append(
            {
                "xp": xp,
                "wq": wlayout(np.asarray(Wq[sl, :], np.float32)),
                "wk": wlayout(np.asarray(Wk[sl, :], np.float32)),
                "wv": wlayout(np.asarray(Wv[sl, :], np.float32)),
                "wout": wout_l,
                "cosf": cos_p,
                "ssinf": ssin_p,
                "qs": np.ascontiguousarray(q_scale, np.float16),
                "ks": np.ascontiguousarray(k_scale, np.float16),
            }
        )
    return in_maps


def kernel(x, rope_emb, Wq, Wk, Wv, Wout, q_scale, k_scale, **run_kwargs):
    in_maps = make_in_maps(
        np.asarray(x, np.float32),
        np.asarray(rope_emb, np.float32),
        np.asarray(Wq, np.float32),
        np.asarray(Wk, np.float32),
        np.asarray(Wv, np.float32),
        np.asarray(Wout, np.float32),
        np.asarray(q_scale, np.float32),
        np.asarray(k_scale, np.float32),
    )
    nc = _get_nc()
    res = run_bass_kernel_spmd(nc, in_maps, core_ids=list(range(NCORES)), **run_kwargs)
    out = np.zeros((S, B, D), dtype=np.float32)
    for c in range(NCORES):
        out[:, c // 4, :] += np.asarray(res.results[c]["out"], np.float32)
    if run_kwargs.get("trace"):
        kernel.last_result = res
    return out
